# revision 10
# baseline (speedup 1.0000x reference)
"""Trainium2 Bass kernel for AdaptiveDiffusionConv (gnn_message_passing).

Reference (per batch b):
    a   = adj * att[b]                      # [m, n]
    out = relu( x@Th0 + a^T (x@Th1 + a^T (x@Th2)) )   (Horner over K=3)

Design:
  * a = adj*att is premultiplied on the HOST and shipped as fp8 (e3m4 —
    best quantization for [0,1] data): 1 byte/elem of DMA and zero
    on-device vector work. The fp8 tiles feed the PE directly as the
    stationary operand (moving side stays bf16; PE upconverts both).
  * Column-block streaming: a arrives as n-class column blocks
    (n = 8q + ci). Block ci completes w's row-block ci immediately, and
    the second hop's rank updates (cj, i') chase the stream.
  * v2 = x@Th2 is computed up front from a host-pre-transposed
    xt[(t,f) rows, (b,i,c,q) cols]; per-tile-pair DMAs so the first
    matmul fires as soon as the first slice lands.
  * Theta is kron(I_6, Th_k) [96,96] in (t,f)/(t,o) order; the w/out
    accumulators for tile i share one 2KB PSUM bank ([w_i | out_i]).
  * Dummy warmup matmuls run during the DMA head so the HAM activity
    monitor lifts the PE cold clock (1.2->2.4GHz) before real work.
  * relu writes res in psum-native (c,t',o) column order, bf16,
    contiguous (strided bf16 writes are slow); host depermutes.
  * Outputs DMA on the GpSimd dynamic queue, inputs on the Sync queue.

Node relabel: m = 8p + j (row tile j, partition p), n = 8q + i (col/out
tile i, partition q), applied consistently everywhere.

Sharding: pure data-parallel over batch B=16 across 8 cores (BL=2).
"""

import sys

sys.path.insert(0, "/opt/trn_rl_repo")

import numpy as np

import concourse.bacc as bacc
import concourse.mybir as mybir
from concourse import tile
from concourse.bass_utils import run_bass_kernel_spmd

B, N, F, T, K, O = 16, 1024, 16, 12, 3, 16
NCORES = 8
BL = B // NCORES  # 2 batches per core
P = 128
NT = N // P  # 8 node tiles
OT = O * T  # 192 cols per tile, (c,t',o) order
HC = 96  # contraction chunk rows (t in 0..5 | 6..11, f); theta block size

F32 = mybir.dt.float32
BF16 = mybir.dt.bfloat16
FP8 = mybir.dt.float8e3  # e3m4: best quantization for [0,1] uniform data
NP_BF16 = mybir.dt.np(BF16)
NP_FP8 = mybir.dt.np(FP8)

WARMUP_MMS = 13  # dummy matmuls bridging the DMA head (HAM warmup)

_CACHE = {}


def build_nc():
    nc = bacc.Bacc()

    a_ext = nc.declare_dram_parameter("a", [BL, NT, P, N], FP8, isOutput=False)
    xt_ext = nc.declare_dram_parameter("xt", [HC, BL * 2 * N], BF16, isOutput=False)
    th_ext = nc.declare_dram_parameter("th", [HC, K * HC], BF16, isOutput=False)
    out_ext = nc.declare_dram_parameter("out", [BL, P, NT, OT], BF16, isOutput=True)

    with tile.TileContext(nc) as tc:
        with (
            tc.tile_pool(name="big", bufs=1) as big,
            tc.tile_pool(name="psp", bufs=8, space="PSUM") as psp,
        ):
            a_sb = big.tile([P, BL * NT * N], FP8)  # a col blocks per batch
            xt_sb = big.tile([HC, BL * 2 * N], BF16)  # cols (b, i, c, q)
            th_sb = big.tile([HC, K * HC], BF16)  # [th2 | th1 | th0]
            vw = big.tile([P, BL * 2 * NT * OT], BF16)  # v2 | w per batch
            res = big.tile([P, BL * NT * OT], BF16)
            wu_sb = big.tile([P, 256], BF16)  # warmup operand (zeros)

            # ---- PE warmup: dummy matmuls fill the DMA head so the HAM
            # activity monitor lifts the 1.2GHz cold-clock before real work
            nc.gpsimd.memset(wu_sb[:], 0.0)
            wu_ps = psp.tile([P, 2 * OT], F32, tag="ps")
            for _ in range(WARMUP_MMS):
                nc.tensor.matmul(
                    wu_ps[:, :256], wu_sb[:, :128], wu_sb[:], start=True, stop=True
                )

            # ---- input DMA: single sync queue in strict priority order —
            # th first (PE rhs gate), xt per tile-pair (v2 starts on first
            # pair), then a blocks (b0 per-block, b1 in halves)
            nc.sync.dma_start(th_sb[:], th_ext[:])
            for b in range(BL):
                for i in range(0, NT, 2):
                    lo = ((b * NT + i) * 2) * P
                    hi = ((b * NT + i + 2) * 2) * P
                    nc.sync.dma_start(xt_sb[:, lo:hi], xt_ext[:, lo:hi])
            for ci in range(NT):
                nc.sync.dma_start(a_sb[:, ci * N : (ci + 1) * N], a_ext[0, ci])
            for h in range(2):
                lo = (NT + h * 4) * N
                hi = (NT + (h + 1) * 4) * N
                nc.sync.dma_start(
                    a_sb[:, lo:hi].rearrange("p (ci n) -> p ci n", ci=4),
                    a_ext[1, h * 4 : (h + 1) * 4].rearrange("ci p n -> p ci n"),
                )

            def a_sl(b, i, j):
                # colblock i of batch b, row tile j: [p, q] = a[8p+j, 8q+i]
                base = (b * NT + i) * N
                return a_sb[:, base + j * P : base + (j + 1) * P]

            def xt_sl(b, i, c):
                base = ((b * NT + i) * 2 + c) * P
                return xt_sb[:, base : base + P]

            def vw_sl(b, s, j):
                base = ((b * 2 + s) * NT + j) * OT
                return vw[:, base : base + OT]

            # ---- v2 = x@Th2: pairs (v2_i | v2_i+1) per bank
            def v2_pair(b, i):
                ps = psp.tile([P, 2 * OT], F32, tag="ps")
                for u in range(2):
                    for c in range(2):
                        nc.tensor.matmul(
                            ps[:, (u * 2 + c) * HC : (u * 2 + c + 1) * HC],
                            xt_sl(b, i + u, c),
                            th_sb[:, :HC],
                            start=(u == 0 and c == 0),
                            stop=(u == 1 and c == 1),
                        )
                nc.scalar.copy(
                    vw[:, (b * 2 * NT + i) * OT : (b * 2 * NT + i + 2) * OT], ps[:]
                )

            def stream(b):
                # 8 banks, bank i = [w_i | out_i]
                pss = []
                for i in range(NT):
                    ps = psp.tile([P, 2 * OT], F32, tag="ps")
                    # open both halves: w_i gets th1, out_i gets th0; the two
                    # matmuls per chunk share the same stationary xt slice
                    for c in range(2):
                        nc.tensor.matmul(
                            ps[:, c * HC : (c + 1) * HC],
                            xt_sl(b, i, c),
                            th_sb[:, HC : 2 * HC],
                            start=(c == 0),
                            stop=False,
                        )
                        nc.tensor.matmul(
                            ps[:, OT + c * HC : OT + (c + 1) * HC],
                            xt_sl(b, i, c),
                            th_sb[:, 2 * HC :],
                            start=False,
                            stop=False,
                        )
                    pss.append(ps)
                # hop-2 rank updates are queued as they become enabled and
                # emitted at most CAP per block, so late blocks' w-loops are
                # not stuck behind the quadratic update backlog; the
                # remainder runs dense after the stream.
                CAP = 6
                pending = []

                def emit_upd(cj, i2, last):
                    nc.tensor.matmul(
                        pss[i2][:, OT:], a_sl(b, i2, cj), vw_sl(b, 1, cj),
                        start=False, stop=last, skip_group_check=True,
                    )

                for ci in range(NT):
                    # hop 1: w row-block ci closes now (stop is sim-only
                    # bookkeeping; it lets the copy read the w half while the
                    # out half keeps accumulating in the same bank)
                    for j in range(NT):
                        nc.tensor.matmul(
                            pss[ci][:, :OT], a_sl(b, ci, j), vw_sl(b, 0, j),
                            start=False, stop=(j == NT - 1),
                        )
                    if b == 1 and ci % 2 == 0:
                        nc.vector.tensor_copy(vw_sl(b, 1, ci)[:], pss[ci][:, :OT])
                    else:
                        nc.scalar.copy(vw_sl(b, 1, ci)[:], pss[ci][:, :OT])
                    pending.extend((cj, ci) for cj in range(ci))
                    pending.append((ci, ci))
                    pending.extend((ci, i2) for i2 in range(ci))
                    take = len(pending) if ci == NT - 1 else min(CAP, len(pending))
                    for _ in range(take):
                        cj, i2 = pending.pop(0)
                        # (cj=7, i2) is always bank i2's final touch
                        emit_upd(cj, i2, cj == NT - 1)
                return pss

            def relus(b, pss):
                # relu + bf16 downcast in psum-native column order (host
                # depermutes); evens on DVE, odds on Act so the chain gating
                # bank reuse / final DMA runs on two engines
                for i2 in range(NT):
                    base = (b * NT + i2) * OT
                    dst = res[:, base : base + OT]
                    if i2 % 2 == 0:
                        nc.vector.tensor_scalar_max(dst, pss[i2][:, OT:], 0.0)
                    else:
                        nc.scalar.activation(
                            dst, pss[i2][:, OT:],
                            mybir.ActivationFunctionType.Relu,
                        )
                    if i2 % 2 == 1:
                        lo = i2 - 1
                        nc.gpsimd.dma_start(
                            out_ext[b][:, lo : i2 + 1, :],
                            res[
                                :, (b * NT + lo) * OT : (b * NT + i2 + 1) * OT
                            ].rearrange("q (r m) -> q r m", r=2),
                        )

            for i in range(0, NT, 2):
                v2_pair(0, i)
            for i in range(0, NT, 2):
                v2_pair(1, i)
            pss0 = stream(0)
            relus(0, pss0)
            pss1 = stream(1)
            relus(1, pss1)

    nc.compile()
    return nc


def make_in_maps(x, att, adj, Theta):
    """Host prep: a=adj*att premultiply, fp8/bf16 casts, layout permutes."""
    x = np.asarray(x, np.float32)
    att = np.asarray(att, np.float32)
    adj = np.asarray(adj, np.float32)
    Theta = np.asarray(Theta, np.float32)

    # a[b] = adj * att[b], then [N,N](m,n) -> [ci, p, (j,q)] with m=8p+j,
    # n=8q+ci, cast to fp8 e3m4
    a_full = (adj[None, :, :] * att).astype(NP_FP8)  # [B, m, n]
    a5 = a_full.reshape(B, P, NT, P, NT)  # [b, p, j, q, ci]
    a_dev = np.ascontiguousarray(a5.transpose(0, 4, 1, 2, 3)).reshape(B, NT, P, N)

    # xt: [(t6,f) rows, (b, i, c, q) cols], n = 8q+i
    xq = x.reshape(B, P, NT, F, T)  # [b, q, i, f, t]
    xt = xq.transpose(0, 2, 4, 3, 1)  # [b, i, t, f, q]
    xt = xt.reshape(B, NT, 2, 6, F, P).reshape(B, NT, 2, 6 * F, P)
    xt = np.ascontiguousarray(xt.transpose(3, 0, 1, 2, 4))  # [96, b, i, c, q]
    xt = xt.reshape(HC, B, 2 * N).astype(NP_BF16)

    th_dev = np.zeros((HC, K * HC), np.float32)
    eye6 = np.eye(6, dtype=np.float32)
    for k in range(K):  # stored order [th2 | th1 | th0]
        th_dev[:, (K - 1 - k) * HC : (K - k) * HC] = np.kron(eye6, Theta[k])
    th_dev = th_dev.astype(NP_BF16)

    in_maps = []
    for c0 in range(NCORES):
        b0 = BL * c0
        in_maps.append(
            {
                "a": np.ascontiguousarray(a_dev[b0 : b0 + BL]),
                "xt": np.ascontiguousarray(xt[:, b0 : b0 + BL].reshape(HC, -1)),
                "th": th_dev,
            }
        )
    return in_maps


def depermute_out(dev_out):
    """Device out [BL, q, i, (c, t', o)] -> [BL, n=8q+i, o, t=6c+t'] f32."""
    o = np.asarray(dev_out).astype(np.float32)
    o = o.reshape(BL, P, NT, 2, 6, O).transpose(0, 1, 2, 5, 3, 4)
    return np.ascontiguousarray(o.reshape(BL, N, O, T))


def kernel(x, spatial_attention, adj, Theta):
    if "nc" not in _CACHE:
        _CACHE["nc"] = build_nc()
    nc = _CACHE["nc"]

    in_maps = make_in_maps(x, spatial_attention, adj, Theta)
    res = run_bass_kernel_spmd(nc, in_maps, core_ids=list(range(NCORES)))
    return np.concatenate(
        [depermute_out(res.results[c]["out"]) for c in range(NCORES)], axis=0
    )


# revision 13
# speedup vs baseline: 1.1762x; 1.1762x over previous
"""Trainium2 Bass kernel for AdaptiveDiffusionConv (gnn_message_passing).

Reference (per batch b):
    a   = adj * att[b]                      # [m, n]
    out = relu( x@Th0 + a^T (x@Th1 + a^T (x@Th2)) )   (Horner over K=3)

Design:
  * a = adj*att is premultiplied on the HOST and shipped as fp8 (e3m4 —
    best quantization for [0,1] data): 1 byte/elem of DMA and zero
    on-device vector work. The fp8 tiles feed the PE directly as the
    stationary operand (moving side stays bf16; PE upconverts both).
  * Column-block streaming: a arrives as n-class column blocks
    (n = 8q + ci). Block ci completes w's row-block ci immediately, and
    the second hop's rank updates (cj, i') chase the stream.
  * v2 = x@Th2 is computed up front from a host-pre-transposed
    xt[(t,f) rows, (b,i,c,q) cols]; per-tile-pair DMAs so the first
    matmul fires as soon as the first slice lands.
  * Theta is kron(I_6, Th_k) [96,96] in (t,f)/(t,o) order; the w/out
    accumulators for tile i share one 2KB PSUM bank ([w_i | out_i]).
  * Dummy warmup matmuls run during the DMA head so the HAM activity
    monitor lifts the PE cold clock (1.2->2.4GHz) before real work.
  * relu writes res in psum-native (c,t',o) column order, bf16,
    contiguous (strided bf16 writes are slow); host depermutes.
  * Outputs DMA on the GpSimd dynamic queue, inputs on the Sync queue.

Node relabel: m = 8p + j (row tile j, partition p), n = 8q + i (col/out
tile i, partition q), applied consistently everywhere.

Sharding: pure data-parallel over batch B=16 across 8 cores (BL=2).
"""

import sys

sys.path.insert(0, "/opt/trn_rl_repo")

import numpy as np

import concourse.bacc as bacc
import concourse.mybir as mybir
from concourse import tile
from concourse.bass_utils import run_bass_kernel_spmd

B, N, F, T, K, O = 16, 1024, 16, 12, 3, 16
NCORES = 8
BL = B // NCORES  # 2 batches per core
P = 128
NT = N // P  # 8 node tiles
OT = O * T  # 192 cols per tile, (c,t',o) order
HC = 96  # contraction chunk rows (t in 0..5 | 6..11, f); theta block size

F32 = mybir.dt.float32
BF16 = mybir.dt.bfloat16
FP8 = mybir.dt.float8e3  # e3m4: best quantization for [0,1] uniform data
NP_BF16 = mybir.dt.np(BF16)
NP_FP8 = mybir.dt.np(FP8)

WARMUP_MMS = 6  # dummy matmuls bridging the DMA head (HAM warmup)

_CACHE = {}


def build_nc():
    nc = bacc.Bacc()

    a_ext = nc.declare_dram_parameter("a", [BL, NT, P, N], FP8, isOutput=False)
    xt_ext = nc.declare_dram_parameter("xt", [HC, BL * 2 * N], BF16, isOutput=False)
    th_ext = nc.declare_dram_parameter("th", [HC, K * HC], BF16, isOutput=False)
    out_ext = nc.declare_dram_parameter("out", [BL, P, NT, OT], BF16, isOutput=True)

    with tile.TileContext(nc) as tc:
        with (
            tc.tile_pool(name="big", bufs=1) as big,
            tc.tile_pool(name="psp", bufs=8, space="PSUM") as psp,
        ):
            a_sb = big.tile([P, BL * NT * N], FP8)  # a col blocks per batch
            xt_sb = big.tile([HC, BL * 2 * N], BF16)  # cols (b, i, c, q)
            th_sb = big.tile([HC, K * HC], BF16)  # [th2 | th1 | th0]
            vw = big.tile([P, BL * 2 * NT * OT], BF16)  # v2 | w per batch
            res = big.tile([P, BL * NT * OT], BF16)
            wu_sb = big.tile([P, 256], BF16)  # warmup operand (zeros)

            # ---- PE warmup: dummy matmuls fill the DMA head so the HAM
            # activity monitor lifts the 1.2GHz cold-clock before real work
            nc.gpsimd.memset(wu_sb[:], 0.0)
            wu_ps = psp.tile([P, 2 * OT], F32, tag="ps")
            for _ in range(WARMUP_MMS):
                nc.tensor.matmul(
                    wu_ps[:, :256], wu_sb[:, :128], wu_sb[:], start=True, stop=True
                )

            # ---- input DMA: single sync queue in strict priority order —
            # th first (PE rhs gate), xt per batch, then a blocks (b0 in
            # pairs, b1 in halves); few big dma_starts — each descriptor
            # write costs ~600ns on the issuing engine
            nc.sync.dma_start(th_sb[:], th_ext[:])
            for b in range(BL):
                nc.sync.dma_start(
                    xt_sb[:, b * 2 * N : (b + 1) * 2 * N],
                    xt_ext[:, b * 2 * N : (b + 1) * 2 * N],
                )
            for g in range(4):
                lo, hi = g * 2 * N, (g + 1) * 2 * N
                nc.sync.dma_start(
                    a_sb[:, lo:hi].rearrange("p (ci n) -> p ci n", ci=2),
                    a_ext[0, g * 2 : (g + 1) * 2].rearrange("ci p n -> p ci n"),
                )
            for h in range(2):
                lo = (NT + h * 4) * N
                hi = (NT + (h + 1) * 4) * N
                nc.sync.dma_start(
                    a_sb[:, lo:hi].rearrange("p (ci n) -> p ci n", ci=4),
                    a_ext[1, h * 4 : (h + 1) * 4].rearrange("ci p n -> p ci n"),
                )

            def a_sl(b, i, j):
                # colblock i of batch b, row tile j: [p, q] = a[8p+j, 8q+i]
                base = (b * NT + i) * N
                return a_sb[:, base + j * P : base + (j + 1) * P]

            def xt_sl(b, i, c):
                base = ((b * NT + i) * 2 + c) * P
                return xt_sb[:, base : base + P]

            def vw_sl(b, s, j):
                base = ((b * 2 + s) * NT + j) * OT
                return vw[:, base : base + OT]

            # ---- v2 = x@Th2: pairs (v2_i | v2_i+1) per bank
            def v2_pair(b, i):
                ps = psp.tile([P, 2 * OT], F32, tag="ps")
                for u in range(2):
                    for c in range(2):
                        nc.tensor.matmul(
                            ps[:, (u * 2 + c) * HC : (u * 2 + c + 1) * HC],
                            xt_sl(b, i + u, c),
                            th_sb[:, :HC],
                            start=(u == 0 and c == 0),
                            stop=(u == 1 and c == 1),
                        )
                nc.scalar.copy(
                    vw[:, (b * 2 * NT + i) * OT : (b * 2 * NT + i + 2) * OT], ps[:]
                )

            def stream(b):
                # 8 banks, bank i = [w_i | out_i]
                pss = []
                for i in range(NT):
                    ps = psp.tile([P, 2 * OT], F32, tag="ps")
                    # open both halves: w_i gets th1, out_i gets th0; the two
                    # matmuls per chunk share the same stationary xt slice
                    for c in range(2):
                        nc.tensor.matmul(
                            ps[:, c * HC : (c + 1) * HC],
                            xt_sl(b, i, c),
                            th_sb[:, HC : 2 * HC],
                            start=(c == 0),
                            stop=False,
                        )
                        nc.tensor.matmul(
                            ps[:, OT + c * HC : OT + (c + 1) * HC],
                            xt_sl(b, i, c),
                            th_sb[:, 2 * HC :],
                            start=False,
                            stop=False,
                        )
                    pss.append(ps)
                # hop-2 rank updates are queued as they become enabled and
                # emitted at most CAP per block, so late blocks' w-loops are
                # not stuck behind the quadratic update backlog; the
                # remainder runs dense after the stream.
                CAP = 6
                pending = []

                def emit_upd(cj, i2, last):
                    nc.tensor.matmul(
                        pss[i2][:, OT:], a_sl(b, i2, cj), vw_sl(b, 1, cj),
                        start=False, stop=last, skip_group_check=True,
                    )

                for ci in range(NT):
                    # hop 1: w row-block ci closes now (stop is sim-only
                    # bookkeeping; it lets the copy read the w half while the
                    # out half keeps accumulating in the same bank)
                    for j in range(NT):
                        nc.tensor.matmul(
                            pss[ci][:, :OT], a_sl(b, ci, j), vw_sl(b, 0, j),
                            start=False, stop=(j == NT - 1),
                        )
                    if b == 1 and ci % 2 == 0:
                        nc.vector.tensor_copy(vw_sl(b, 1, ci)[:], pss[ci][:, :OT])
                    else:
                        nc.scalar.copy(vw_sl(b, 1, ci)[:], pss[ci][:, :OT])
                    pending.extend((cj, ci) for cj in range(ci))
                    pending.append((ci, ci))
                    pending.extend((ci, i2) for i2 in range(ci))
                    take = len(pending) if ci == NT - 1 else min(CAP, len(pending))
                    for _ in range(take):
                        cj, i2 = pending.pop(0)
                        # (cj=7, i2) is always bank i2's final touch
                        emit_upd(cj, i2, cj == NT - 1)
                return pss

            def relus(b, pss):
                # relu + bf16 downcast in psum-native column order (host
                # depermutes); evens on DVE, odds on Act so the chain gating
                # bank reuse / final DMA runs on two engines
                for i2 in range(NT):
                    base = (b * NT + i2) * OT
                    dst = res[:, base : base + OT]
                    if i2 % 2 == 0:
                        nc.vector.tensor_scalar_max(dst, pss[i2][:, OT:], 0.0)
                    else:
                        nc.scalar.activation(
                            dst, pss[i2][:, OT:],
                            mybir.ActivationFunctionType.Relu,
                        )
                    if i2 % 2 == 1:
                        lo = i2 - 1
                        nc.sync.dma_start(
                            out_ext[b][:, lo : i2 + 1, :],
                            res[
                                :, (b * NT + lo) * OT : (b * NT + i2 + 1) * OT
                            ].rearrange("q (r m) -> q r m", r=2),
                        )

            for i in range(0, NT, 2):
                v2_pair(0, i)
            for i in range(0, NT, 2):
                v2_pair(1, i)
            pss0 = stream(0)
            relus(0, pss0)
            pss1 = stream(1)
            relus(1, pss1)

    nc.compile()
    return nc


def make_in_maps(x, att, adj, Theta):
    """Host prep: a=adj*att premultiply, fp8/bf16 casts, layout permutes."""
    x = np.asarray(x, np.float32)
    att = np.asarray(att, np.float32)
    adj = np.asarray(adj, np.float32)
    Theta = np.asarray(Theta, np.float32)

    # a[b] = adj * att[b], then [N,N](m,n) -> [ci, p, (j,q)] with m=8p+j,
    # n=8q+ci, cast to fp8 e3m4
    a_full = (adj[None, :, :] * att).astype(NP_FP8)  # [B, m, n]
    a5 = a_full.reshape(B, P, NT, P, NT)  # [b, p, j, q, ci]
    a_dev = np.ascontiguousarray(a5.transpose(0, 4, 1, 2, 3)).reshape(B, NT, P, N)

    # xt: [(t6,f) rows, (b, i, c, q) cols], n = 8q+i
    xq = x.reshape(B, P, NT, F, T)  # [b, q, i, f, t]
    xt = xq.transpose(0, 2, 4, 3, 1)  # [b, i, t, f, q]
    xt = xt.reshape(B, NT, 2, 6, F, P).reshape(B, NT, 2, 6 * F, P)
    xt = np.ascontiguousarray(xt.transpose(3, 0, 1, 2, 4))  # [96, b, i, c, q]
    xt = xt.reshape(HC, B, 2 * N).astype(NP_BF16)

    th_dev = np.zeros((HC, K * HC), np.float32)
    eye6 = np.eye(6, dtype=np.float32)
    for k in range(K):  # stored order [th2 | th1 | th0]
        th_dev[:, (K - 1 - k) * HC : (K - k) * HC] = np.kron(eye6, Theta[k])
    th_dev = th_dev.astype(NP_BF16)

    in_maps = []
    for c0 in range(NCORES):
        b0 = BL * c0
        in_maps.append(
            {
                "a": np.ascontiguousarray(a_dev[b0 : b0 + BL]),
                "xt": np.ascontiguousarray(xt[:, b0 : b0 + BL].reshape(HC, -1)),
                "th": th_dev,
            }
        )
    return in_maps


def depermute_out(dev_out):
    """Device out [BL, q, i, (c, t', o)] -> [BL, n=8q+i, o, t=6c+t'] f32."""
    o = np.asarray(dev_out).astype(np.float32)
    o = o.reshape(BL, P, NT, 2, 6, O).transpose(0, 1, 2, 5, 3, 4)
    return np.ascontiguousarray(o.reshape(BL, N, O, T))


def kernel(x, spatial_attention, adj, Theta):
    if "nc" not in _CACHE:
        _CACHE["nc"] = build_nc()
    nc = _CACHE["nc"]

    in_maps = make_in_maps(x, spatial_attention, adj, Theta)
    res = run_bass_kernel_spmd(nc, in_maps, core_ids=list(range(NCORES)))
    return np.concatenate(
        [depermute_out(res.results[c]["out"]) for c in range(NCORES)], axis=0
    )
